# revision 1
# baseline (speedup 1.0000x reference)
"""Pairwise L2-distance kernel (retrieval_knn) for 8x Trainium2 NeuronCores.

Computes Z = beta - sqrt(max(||x||^2 + ||y||^2 - 2 X@Y, 0)) for
X:(8192,256) f32, Y:(256,8192) f32, beta:(1,) f32 -> Z:(8192,8192) f32.

Sharding: X row-wise across 8 cores (1024 rows each); Y and beta replicated.
Each core computes a (1024, 8192) slab of Z; the host concatenates slabs.

Per-core algorithm:
  - cross term via PE matmul in bf16 with X scaled by -2 at convert time
    (exact power-of-two scaling), K=256 split into 2 chunks of 128.
  - ||y||^2 injected into the same PSUM accumulation as one extra fp16
    contraction row (ones (x) y2_fp16); fp16 keeps y2's absolute error
    ~0.12 on values ~512 (vs ~1-2 for bf16).
  - ||x||^2 (exact fp32) added via the per-partition bias of the fused
    ScalarE Sqrt activation reading PSUM.
  - z = beta - d in a single VectorE tensor_scalar pass: (d * -1) + beta.
"""

from contextlib import ExitStack

import numpy as np

import concourse.bacc as bacc
import concourse.mybir as mybir
import concourse.tile as tile
from concourse.bass_utils import run_bass_kernel_spmd
from concourse.masks import make_identity

N_CORES = 8
N_ROW, RANK, N_COL = 8192, 256, 8192
ROWS_PER_CORE = N_ROW // N_CORES  # 1024

P = 128      # partitions
FN = 512     # matmul free dim / PSUM bank (fp32)

f32 = mybir.dt.float32
bf16 = mybir.dt.bfloat16
f16 = mybir.dt.float16

AF = mybir.ActivationFunctionType
ALU = mybir.AluOpType


def build_l2_kernel(rows=ROWS_PER_CORE, rank=RANK, ncol=N_COL, n_cores=N_CORES,
                    out_bufs=8, psum_bufs=6,
                    use_e_row=True, use_beta_ap=True, use_x_side=True,
                    use_y_side=True, use_main=True,
                    use_x2=True, use_xT=True):
    """Build the per-core SPMD Bass program. Returns the compiled Bacc."""
    assert rows % P == 0 and rank % P == 0 and ncol % FN == 0
    mt = rows // P          # m-tiles (8)
    kc = rank // P          # k-chunks (2)
    nt = ncol // FN         # n-tiles (16)

    nc = bacc.Bacc("TRN2", target_bir_lowering=False, debug=False,
                   num_devices=n_cores)

    xs_d = nc.dram_tensor("Xs", [rows, rank], f32, kind="ExternalInput")
    y_d = nc.dram_tensor("Y", [rank, ncol], f32, kind="ExternalInput")
    beta_d = nc.dram_tensor("beta", [1, 1], f32, kind="ExternalInput")
    # Z stored as [mt, nt, 128, 512] tile blocks -> every DMA store is one
    # fully contiguous 256KB burst. Host reassembles to [rows, ncol].
    z_d = nc.dram_tensor("Z", [mt * nt * P, FN], f32, kind="ExternalOutput")

    with tile.TileContext(nc) as tc, ExitStack() as ctx:
        cpool = ctx.enter_context(tc.tile_pool(name="const", bufs=1))
        ypool = ctx.enter_context(tc.tile_pool(name="ybig", bufs=1))
        yf_pool = ctx.enter_context(tc.tile_pool(name="yf", bufs=6))
        ysq_pool = ctx.enter_context(tc.tile_pool(name="ysq", bufs=6))
        setup_psum = ExitStack()
        tp_psum = setup_psum.enter_context(
            tc.tile_pool(name="tpp", bufs=2, space="PSUM"))
        y2_psum = setup_psum.enter_context(
            tc.tile_pool(name="y2p", bufs=2, space="PSUM"))
        dpool = ctx.enter_context(tc.tile_pool(name="d", bufs=out_bufs))

        # ---- constants ----
        identity = cpool.tile([P, P], f32)
        make_identity(nc, identity[:])
        ones_row = cpool.tile([1, P], f16)       # lhsT of the y2-row matmul
        nc.gpsimd.memset(ones_row[:], 1.0)
        ones_col = cpool.tile([P, 1], bf16)      # lhsT of the y2 column-reduce
        nc.gpsimd.memset(ones_col[:], 1.0)
        if use_beta_ap:
            beta_b = cpool.tile([P, 1], f32)
            b11 = cpool.tile([1, 1], f32)
            nc.sync.dma_start(b11[:], beta_d.ap()[:])
            nc.gpsimd.partition_broadcast(beta_b[:], b11[:])
        else:
            beta_b = None

        # ---- X side: load slab, x2, transposed -2X in bf16 ----
        xs_sb = cpool.tile([P, mt, rank], f32)
        nc.sync.dma_start(
            xs_sb[:], xs_d.ap().rearrange("(t p) k -> p t k", p=P))

        x2 = cpool.tile([P, mt], f32)
        xsq = cpool.tile([P, rank], f32)
        xbT = [cpool.tile([P, rows], bf16, name=f"xbT{c}", tag=f"xbT{c}")
               for c in range(kc)]
        for m in range(mt if use_x_side else 0):
            if use_x2:
                nc.vector.tensor_tensor(
                    xsq[:], xs_sb[:, m, :], xs_sb[:, m, :], op=ALU.mult)
                nc.vector.reduce_sum(
                    x2[:, m : m + 1], xsq[:], axis=mybir.AxisListType.X)
            for c in range(kc if use_xT else 0):
                pt = tp_psum.tile([P, P], f32)
                nc.tensor.transpose(
                    pt[:], xs_sb[:, m, c * P : (c + 1) * P], identity[:])
                nc.scalar.activation(
                    xbT[c][:, m * P : (m + 1) * P], pt[:],
                    AF.Copy, scale=-2.0)

        # ---- Y side: stream pieces, convert to bf16, y2 -> fp16 row ----
        yb = [ypool.tile([P, ncol], bf16, name=f"yb{c}", tag=f"yb{c}")
              for c in range(kc)]
        e_row = cpool.tile([1, ncol], f16)
        for j in range(nt if use_y_side else 0):
            y2ps = y2_psum.tile([1, FN], f32)
            for c in range(kc):
                yf = yf_pool.tile([P, FN], f32)
                nc.sync.dma_start(
                    yf[:], y_d.ap()[c * P : (c + 1) * P,
                                    j * FN : (j + 1) * FN])
                nc.vector.tensor_copy(yb[c][:, j * FN : (j + 1) * FN], yf[:])
                ysq = ysq_pool.tile([P, FN], bf16)
                nc.scalar.activation(ysq[:], yf[:], AF.Square)
                nc.tensor.matmul(
                    y2ps[:], ones_col[:], ysq[:],
                    start=(c == 0), stop=(c == kc - 1))
            if use_e_row:
                nc.scalar.activation(
                    e_row[:, j * FN : (j + 1) * FN], y2ps[:], AF.Copy)

        # ---- main loop ----
        # setup PSUM pools released here -> main matmuls get 6 banks
        setup_psum.close()
        mm_psum = ctx.enter_context(
            tc.tile_pool(name="mmp", bufs=psum_bufs, space="PSUM"))
        for m in range(mt if (use_main and use_x_side and use_y_side) else 0):
            for j in range(nt):
                ps = mm_psum.tile([P, FN], f32)
                for c in range(kc):
                    nc.tensor.matmul(
                        ps[:], xbT[c][:, m * P : (m + 1) * P],
                        yb[c][:, j * FN : (j + 1) * FN],
                        start=(c == 0), stop=(not use_e_row and c == kc - 1))
                if use_e_row:
                    nc.tensor.matmul(
                        ps[:], ones_row[:],
                        e_row[:, j * FN : (j + 1) * FN],
                        start=False, stop=True)
                d = dpool.tile([P, FN], f32)
                nc.scalar.activation(
                    d[:], ps[:], AF.Sqrt, bias=x2[:, m : m + 1])
                nc.vector.tensor_scalar(
                    d[:], d[:], -1.0,
                    beta_b[:] if use_beta_ap else 0.0, ALU.mult, ALU.add)
                blk = (m * nt + j) * P
                nc.sync.dma_start(z_d.ap()[blk : blk + P, :], d[:])

    nc.compile()
    return nc


_CACHED = {}


def _get_nc():
    if "nc" not in _CACHED:
        _CACHED["nc"] = build_l2_kernel()
    return _CACHED["nc"]


def kernel(X, Y, beta):
    X = np.ascontiguousarray(np.asarray(X, dtype=np.float32))
    Y = np.ascontiguousarray(np.asarray(Y, dtype=np.float32))
    beta = np.asarray(beta, dtype=np.float32).reshape(1, 1)
    assert X.shape == (N_ROW, RANK) and Y.shape == (RANK, N_COL)

    nc = _get_nc()
    in_maps = [
        {"Xs": X[c * ROWS_PER_CORE : (c + 1) * ROWS_PER_CORE], "Y": Y,
         "beta": beta}
        for c in range(N_CORES)
    ]
    res = run_bass_kernel_spmd(nc, in_maps, core_ids=list(range(N_CORES)))
    mt, nt = ROWS_PER_CORE // P, N_COL // FN
    slabs = [
        res.results[c]["Z"].reshape(mt, nt, P, FN)
        .transpose(0, 2, 1, 3).reshape(ROWS_PER_CORE, N_COL)
        for c in range(N_CORES)
    ]
    return np.ascontiguousarray(np.concatenate(slabs, axis=0))



# revision 2
# speedup vs baseline: 16.5687x; 16.5687x over previous
"""Pairwise L2-distance kernel (retrieval_knn) for 8x Trainium2 NeuronCores.

Computes Z = beta - sqrt(max(||x||^2 + ||y||^2 - 2 X@Y, 0)) for
X:(8192,256) f32, Y:(256,8192) f32, beta:(1,) f32 -> Z:(8192,8192) f32.

Sharding: X row-wise across 8 cores (1024 rows each); Y and beta replicated.
Each core computes a (1024, 8192) slab of Z; the host concatenates slabs.

Per-core algorithm (V1):
  - cross term via PE matmul in bf16 with X scaled by -2 at convert time
    (exact power-of-two scaling), K=256 split into 2 chunks of 128.
  - ||y||^2 injected into the same PSUM accumulation as one extra fp16
    contraction row (ones (x) y2_fp16).
  - ||x||^2 (exact fp32) added via the per-partition bias of the fused
    ScalarE Sqrt activation reading PSUM; output is fp16 (tolerance is
    2e-2; fp16 rounding adds ~3e-4).
  - z = beta - d in a single VectorE tensor_scalar pass, fp16 in/out and
    all-SBUF so the DVE runs in 4x mode.
  - Act/DVE operate on [128, 2048] chunks (4 PSUM banks) to amortize
    access latency and instruction overhead; Z is stored fp16 with one
    2MB DMA per 128-row block and converted to f32 on the host.
"""

from contextlib import ExitStack

import numpy as np

import concourse.bacc as bacc
import concourse.mybir as mybir
import concourse.tile as tile
from concourse.bass_utils import run_bass_kernel_spmd
from concourse.masks import make_identity

N_CORES = 8
N_ROW, RANK, N_COL = 8192, 256, 8192
ROWS_PER_CORE = N_ROW // N_CORES  # 1024

P = 128      # partitions
FN = 512     # matmul free dim / PSUM bank (fp32)
CH = 2048    # Act/DVE chunk width (4 PSUM banks)

f32 = mybir.dt.float32
bf16 = mybir.dt.bfloat16
f16 = mybir.dt.float16

AF = mybir.ActivationFunctionType
ALU = mybir.AluOpType


def build_l2_kernel(rows=ROWS_PER_CORE, rank=RANK, ncol=N_COL, n_cores=N_CORES):
    """Build the per-core SPMD Bass program. Returns the compiled Bacc."""
    assert rows % P == 0 and rank % P == 0 and ncol % CH == 0
    mt = rows // P          # m-tiles (8)
    kc = rank // P          # k-chunks (2)
    ng = ncol // CH         # n-chunks (4)
    spc = CH // FN          # psum banks (512-wide subtiles) per chunk (4)

    nc = bacc.Bacc("TRN2", target_bir_lowering=False, debug=False,
                   num_devices=n_cores)

    xs_d = nc.dram_tensor("Xs", [rows, rank], f32, kind="ExternalInput")
    y_d = nc.dram_tensor("Y", [rank, ncol], f32, kind="ExternalInput")
    beta_d = nc.dram_tensor("beta", [1, 1], f32, kind="ExternalInput")
    # fp16 output slab; host converts to f32.
    z_d = nc.dram_tensor("Z", [rows, ncol], f16, kind="ExternalOutput")

    with tile.TileContext(nc) as tc, ExitStack() as ctx:
        cpool = ctx.enter_context(tc.tile_pool(name="const", bufs=1))
        ypool = ctx.enter_context(tc.tile_pool(name="ybig", bufs=1))
        yf_pool = ctx.enter_context(tc.tile_pool(name="yf", bufs=3))
        ysq_pool = ctx.enter_context(tc.tile_pool(name="ysq", bufs=3))
        setup_psum = ExitStack()
        tp_psum = setup_psum.enter_context(
            tc.tile_pool(name="tpp", bufs=2, space="PSUM"))
        y2_psum = setup_psum.enter_context(
            tc.tile_pool(name="y2p", bufs=1, space="PSUM"))
        zpool = ctx.enter_context(tc.tile_pool(name="z", bufs=2))

        # ---- constants ----
        identity = cpool.tile([P, P], f32)
        make_identity(nc, identity[:])
        ones_row = cpool.tile([1, P], f16)       # lhsT of the y2-row matmul
        nc.gpsimd.memset(ones_row[:], 1.0)
        ones_col = cpool.tile([P, 1], bf16)      # lhsT of the y2 column-reduce
        nc.gpsimd.memset(ones_col[:], 1.0)
        beta_b = cpool.tile([P, 1], f32)
        b11 = cpool.tile([1, 1], f32)
        nc.sync.dma_start(b11[:], beta_d.ap()[:])
        nc.gpsimd.partition_broadcast(beta_b[:], b11[:])

        # ---- X side: load slab, x2, transposed -2X in bf16 ----
        xs_sb = cpool.tile([P, mt, rank], f32)
        nc.sync.dma_start(
            xs_sb[:], xs_d.ap().rearrange("(t p) k -> p t k", p=P))

        x2 = cpool.tile([P, mt], f32)
        xsq = cpool.tile([P, rank], f32)
        xbT = [cpool.tile([P, rows], bf16, name=f"xbT{c}", tag=f"xbT{c}")
               for c in range(kc)]
        for m in range(mt):
            nc.vector.tensor_tensor(
                xsq[:], xs_sb[:, m, :], xs_sb[:, m, :], op=ALU.mult)
            nc.vector.reduce_sum(
                x2[:, m : m + 1], xsq[:], axis=mybir.AxisListType.X)
            for c in range(kc):
                pt = tp_psum.tile([P, P], f32)
                nc.tensor.transpose(
                    pt[:], xs_sb[:, m, c * P : (c + 1) * P], identity[:])
                nc.scalar.activation(
                    xbT[c][:, m * P : (m + 1) * P], pt[:],
                    AF.Copy, scale=-2.0)

        # ---- Y side: stream 2048-wide pieces, convert to bf16, y2 row ----
        yb = [ypool.tile([P, ncol], bf16, name=f"yb{c}", tag=f"yb{c}")
              for c in range(kc)]
        e_row = cpool.tile([1, ncol], f16)
        for g in range(ng):
            y2ps = y2_psum.tile([1, CH], f32)
            for c in range(kc):
                yf = yf_pool.tile([P, CH], f32)
                nc.sync.dma_start(
                    yf[:], y_d.ap()[c * P : (c + 1) * P,
                                    g * CH : (g + 1) * CH])
                nc.vector.tensor_copy(yb[c][:, g * CH : (g + 1) * CH], yf[:])
                ysq = ysq_pool.tile([P, CH], bf16)
                nc.scalar.activation(ysq[:], yf[:], AF.Square)
                for s in range(spc):
                    nc.tensor.matmul(
                        y2ps[:, s * FN : (s + 1) * FN], ones_col[:],
                        ysq[:, s * FN : (s + 1) * FN],
                        start=(c == 0), stop=(c == kc - 1))
            nc.scalar.activation(
                e_row[:, g * CH : (g + 1) * CH], y2ps[:], AF.Copy)

        # ---- main loop ----
        # setup PSUM pools released here -> main matmuls get all 8 banks
        setup_psum.close()
        mm_psum = ctx.enter_context(
            tc.tile_pool(name="mmp", bufs=2, space="PSUM"))
        for m in range(mt):
            zrow = zpool.tile([P, ncol], f16)
            for g in range(ng):
                ps = mm_psum.tile([P, CH], f32)
                for s in range(spc):
                    lo = g * CH + s * FN
                    pss = ps[:, s * FN : (s + 1) * FN]
                    for c in range(kc):
                        nc.tensor.matmul(
                            pss, xbT[c][:, m * P : (m + 1) * P],
                            yb[c][:, lo : lo + FN],
                            start=(c == 0), stop=False)
                    nc.tensor.matmul(
                        pss, ones_row[:], e_row[:, lo : lo + FN],
                        start=False, stop=True)
                zr = zrow[:, g * CH : (g + 1) * CH]
                nc.scalar.activation(zr, ps[:], AF.Sqrt,
                                     bias=x2[:, m : m + 1])
                nc.vector.tensor_scalar(
                    zr, zr, -1.0, beta_b[:], ALU.mult, ALU.add)
            nc.sync.dma_start(
                z_d.ap()[m * P : (m + 1) * P, :], zrow[:])

    nc.compile()
    return nc


_CACHED = {}


def _get_nc():
    if "nc" not in _CACHED:
        _CACHED["nc"] = build_l2_kernel()
    return _CACHED["nc"]


def kernel(X, Y, beta):
    X = np.ascontiguousarray(np.asarray(X, dtype=np.float32))
    Y = np.ascontiguousarray(np.asarray(Y, dtype=np.float32))
    beta = np.asarray(beta, dtype=np.float32).reshape(1, 1)
    assert X.shape == (N_ROW, RANK) and Y.shape == (RANK, N_COL)

    nc = _get_nc()
    in_maps = [
        {"Xs": X[c * ROWS_PER_CORE : (c + 1) * ROWS_PER_CORE], "Y": Y,
         "beta": beta}
        for c in range(N_CORES)
    ]
    res = run_bass_kernel_spmd(nc, in_maps, core_ids=list(range(N_CORES)))
    slabs = [res.results[c]["Z"].astype(np.float32) for c in range(N_CORES)]
    return np.ascontiguousarray(np.concatenate(slabs, axis=0))


# revision 39
# speedup vs baseline: 28.7186x; 1.7333x over previous
"""Pairwise L2-distance kernel (retrieval_knn) for 8x Trainium2 NeuronCores.

Computes Z = beta - sqrt(max(||x||^2 + ||y||^2 - 2 X@Y, 0)) for
X:(8192,256) f32, Y:(256,8192) f32, beta:(1,) f32 -> Z:(8192,8192) f32.

Sharding: 2D grid, 4 row-blocks x 2 col-blocks. Core c handles
X rows [2048*(c//2), +2048) and Y cols [4096*(c%2), +4096), writing a
(2048, 4096) block of Z. Vs 1D row sharding this cuts per-core input
DMA from 9.4MB to 6.3MB and halves the Y-side prologue.

Per-core algorithm (V7):
  - cross term via one fp8e4 DoubleRow PE matmul per 512-wide subtile
    (K=256 as 128 partition pairs), X scaled by -2 at convert time.
    fp8 noise contributes ~1.2e-3 relative error (tolerance 2e-2).
    use_fp8=False falls back to bf16 2-pass (more PE cycles, ~5e-5).
  - ||y||^2 injected into each PSUM accumulation group as one extra f16
    K=1 contraction row (ones (x) y2_f16), issued first (start=True).
  - main loop is m-major with 2048-wide groups; each LDWEIGHTS covers 4
    back-to-back matmuls and 2 PSUM rounds are in flight — this exact
    structure measured HAM-warm (2.4 GHz); narrower variants throttle.
  - zero-weight filler matmuls (lhsT=0, adds 0 to PSUM) pad the PE so
    it stays just above ScalarE's chunk rate and the HAM clock gate
    never re-throttles; without them the fp8 PE idles between groups
    and drops to 1.2 GHz, costing more than the fillers (V2/V3).
  - Y-side streaming interleaved into the m=0 iteration; y2 sums borrow
    row 0 of the m=0 PSUM tiles (no extra PSUM banks).
  - ||x||^2 via Square+accum_out on the otherwise-idle ScalarE in the
    prologue, applied as the per-partition bias of the fused Sqrt
    activation reading 4 PSUM banks at once; fp16 output.
  - z = beta - d in one VectorE tensor_scalar pass (fp16, 4x DVE mode).
  - output staged in per-group fp16 tiles, each stored with its own
    DMA; host converts to f32 and assembles the 2D blocks.
"""

from contextlib import ExitStack

import numpy as np

import concourse.bacc as bacc
import concourse.mybir as mybir
import concourse.tile as tile
from concourse.bass_utils import run_bass_kernel_spmd
from concourse.masks import make_identity

N_CORES = 8
N_ROW, RANK, N_COL = 8192, 256, 8192
R_BLK, C_BLK = 4, 2                    # core grid
ROWS_PER_CORE = N_ROW // R_BLK         # 2048
COLS_PER_CORE = N_COL // C_BLK         # 4096

P = 128      # partitions
FN = 512     # matmul free dim / PSUM bank (fp32)
CH = 1024    # Y-side strip width
GP = 2048    # main group width (4 PSUM banks; Act/DVE/store unit)

f32 = mybir.dt.float32
bf16 = mybir.dt.bfloat16
f16 = mybir.dt.float16
f8 = mybir.dt.float8e4

AF = mybir.ActivationFunctionType
ALU = mybir.AluOpType
DRMODE = mybir.MatmulPerfMode.DoubleRow


def build_l2_kernel(rows=ROWS_PER_CORE, rank=RANK, ncol=COLS_PER_CORE,
                    n_cores=N_CORES, use_fp8=False, fillers=0):
    """Build the per-core SPMD Bass program. Returns the compiled Bacc."""
    assert rows % P == 0 and rank % P == 0 and ncol % GP == 0
    mt = rows // P          # m-tiles (16)
    kc = rank // P          # k-chunks (2)
    ngp = ncol // GP        # groups per m row (2)
    spg = GP // FN          # 512-subtiles per group (4)

    nc = bacc.Bacc("TRN2", target_bir_lowering=False, debug=False,
                   num_devices=n_cores)

    xs_d = nc.dram_tensor("Xs", [rows, rank], f32, kind="ExternalInput")
    y_d = nc.dram_tensor("Y", [rank, ncol], f32, kind="ExternalInput")
    beta_d = nc.dram_tensor("beta", [1, 1], f32, kind="ExternalInput")
    # fp16 output block; host converts to f32.
    z_d = nc.dram_tensor("Z", [rows, ncol], f16, kind="ExternalOutput")

    with tile.TileContext(nc) as tc, ExitStack() as ctx:
        cpool = ctx.enter_context(tc.tile_pool(name="const", bufs=1))
        ypool = ctx.enter_context(tc.tile_pool(name="ybig", bufs=1))
        yf_pool = ctx.enter_context(tc.tile_pool(name="yf", bufs=8))
        ysq_pool = ctx.enter_context(tc.tile_pool(name="ysq", bufs=4))
        zpool = ctx.enter_context(tc.tile_pool(name="z", bufs=4))
        tp_stack = ExitStack()
        tp_psum = tp_stack.enter_context(
            tc.tile_pool(name="tpp", bufs=4, space="PSUM"))

        # ---- constants ----
        identity = cpool.tile([P, P], f32)
        make_identity(nc, identity[:])
        ones_row = cpool.tile([1, P], f16)       # lhsT of the y2-row matmul
        nc.gpsimd.memset(ones_row[:], 1.0)
        zeros_row = cpool.tile([1, P], f16)      # lhsT of filler matmuls
        nc.gpsimd.memset(zeros_row[:], 0.0)
        ones_col = cpool.tile([P, 1], bf16)      # lhsT of the y2 column-reduce
        nc.gpsimd.memset(ones_col[:], 1.0)
        beta_b = cpool.tile([P, 1], f32)
        b11 = cpool.tile([1, 1], f32)
        nc.sync.dma_start(b11[:], beta_d.ap()[:])
        nc.gpsimd.partition_broadcast(beta_b[:], b11[:])

        # ---- DMA order: Y strip 0 first (it gates the PE's first work),
        # then X, then the rest of Y. All loads upfront: the SP DMA queue
        # is in-order, so a load emitted after the main loop's stores
        # would not fire until every earlier store's data landed.
        yfs = {}

        def y_load(g):
            for c in range(kc):
                yf = yf_pool.tile([P, CH], f32, name="yf")
                nc.sync.dma_start(
                    yf[:], y_d.ap()[c * P : (c + 1) * P,
                                    g * CH : (g + 1) * CH])
                yfs[(g, c)] = yf

        y_load(0)
        xs_sb = cpool.tile([P, mt, rank], f32)
        nc.sync.dma_start(
            xs_sb[:], xs_d.ap().rearrange("(t p) k -> p t k", p=P))
        for g in range(1, ncol // CH):
            y_load(g)

        yT = ypool.tile([P, kc, ncol], f8 if use_fp8 else bf16,
                        name="yb", tag="yb")
        e_row = cpool.tile([1, ncol], f16)

        def y_conv(g):
            """bf16 conversion of strip g on ScalarE."""
            lo0 = g * CH
            for c in range(kc):
                nc.scalar.copy(yT[:, c, lo0 : lo0 + CH], yfs[(g, c)][:])

        def y_sq(g):
            """y^2 of strip g on the DVE (from the f32 staging tiles)."""
            sqs = []
            for c in range(kc):
                ysq = ysq_pool.tile([P, CH], bf16, name="ysq")
                nc.vector.tensor_tensor(
                    ysq[:], yfs[(g, c)][:], yfs[(g, c)][:], op=ALU.mult)
                sqs.append(ysq)
            return sqs

        def y_sum(g, sqs, y2ps):
            """Column-sum y^2 on the PE into the borrowed [1, CH] PSUM
            row (e_copy emitted separately)."""
            for c in range(kc):
                for s in range(CH // FN):
                    nc.tensor.matmul(
                        y2ps[0:1, s * FN : (s + 1) * FN], ones_col[:],
                        sqs[c][:, s * FN : (s + 1) * FN],
                        start=(c == 0), stop=(c == kc - 1))

        def e_copy(g, y2ps):
            nc.vector.tensor_copy(
                e_row[:, g * CH : (g + 1) * CH], y2ps[0:1, :])

        x2 = cpool.tile([P, mt], f32)
        xsq = cpool.tile([P, rank], f32)
        xT = cpool.tile([P, kc, rows], f8 if use_fp8 else bf16)

        # Emission order tuned so no engine's in-order queue blocks
        # another's first need:
        #  PE:  y2sum(s0) ~11us -> 32 transposes (DVE-paced) -> y2sum(s1)
        #       -> main matmuls, continuous => HAM warms once, stays warm
        #  Act: conv(s0) conv(s1) x2*16 -> sqrt(m0) ready ~13us
        #  DVE: ysq(s0) -> xT converts (paces transposes) -> ysq(s1) ->
        #       beta passes
        y2p_stack = ExitStack()
        y2_psum = y2p_stack.enter_context(
            tc.tile_pool(name="y2p", bufs=1, space="PSUM"))
        y2t = y2_psum.tile([1, GP], f32)

        # HAM warm-up: ~14 back-to-back matmuls that depend only on
        # constants, so the PE starts streaming at ~7us (right after
        # NEFF init) and the clock gate reaches 2.4 GHz before the real
        # work begins. Results land in a y2t slice that y_sum overwrites
        # (start=True) later. Only sustained (>3.4us) idleness
        # re-throttles, which the pipeline below avoids.
        warm_src = cpool.tile([P, FN], bf16)
        nc.vector.memset(warm_src[:], 0.0)

        def warm(n):
            for _ in range(n):
                nc.tensor.matmul(y2t[0:1, 0:FN], ones_col[:],
                                 warm_src[:], start=True, stop=True)

        warm(14)

        def transposes(m_lo, m_hi):
            for m in range(m_lo, m_hi):
                for c in range(kc):
                    pt = tp_psum.tile([P, P], f32, name="pt")
                    nc.tensor.transpose(
                        pt[:], xs_sb[:, m, c * P : (c + 1) * P],
                        identity[:])
                    nc.vector.tensor_scalar(
                        xT[:, c, m * P : (m + 1) * P], pt[:], -2.0, 0.0,
                        ALU.mult, ALU.add)

        y_conv(0)
        y_conv(1)
        s0 = y_sq(0)
        s1 = y_sq(1)
        y_sum(0, s0, y2t[0:1, 0:CH])
        transposes(0, 4)
        e_copy(0, y2t[0:1, 0:CH])
        y_sum(1, s1, y2t[0:1, CH : 2 * CH])
        transposes(4, 8)
        e_copy(1, y2t[0:1, CH : 2 * CH])
        transposes(8, mt)
        for m in range(mt):
            # x2 via Square+accum_out on ScalarE (idle in the prologue)
            nc.scalar.activation(
                xsq[:], xs_sb[:, m, :], AF.Square,
                accum_out=x2[:, m : m + 1])
        y2p_stack.close()
        tp_stack.close()
        mm_psum = ctx.enter_context(
            tc.tile_pool(name="mmp", bufs=2, space="PSUM"))

        # ---- main loop: g-outer / m-inner. Column g's strips are ready
        # before it starts; the NEXT column's strips are processed early
        # (spread over m=2..6, borrowing PSUM rows at m=6) so the column
        # boundary has no Y-chain stall and the PE never idles.
        for g in range(ngp):
            for m in range(mt):
                ps = mm_psum.tile([P, GP], f32)
                if g + 1 < ngp:
                    if m == 2:
                        y_conv(2 * (g + 1))
                        nxt_s0 = y_sq(2 * (g + 1))
                    elif m == 4:
                        y_conv(2 * (g + 1) + 1)
                        nxt_s1 = y_sq(2 * (g + 1) + 1)
                    elif m == 6:
                        y_sum(2 * (g + 1), nxt_s0, ps[:, 0:CH])
                        e_copy(2 * (g + 1), ps[:, 0:CH])
                    elif m == 8:
                        y_sum(2 * (g + 1) + 1, nxt_s1, ps[:, 0:CH])
                        e_copy(2 * (g + 1) + 1, ps[:, 0:CH])

                def sub(s):
                    return ps[:, s * FN : (s + 1) * FN]

                # y2 row first (start=True); one LDWEIGHTS covers 4 MMs
                for s in range(spg):
                    lo = g * GP + s * FN
                    nc.tensor.matmul(
                        sub(s), ones_row[:], e_row[:, lo : lo + FN],
                        start=True, stop=False)
                # zero-weight fillers: add 0, keep the PE HAM-warm
                for f in range(fillers):
                    lo = g * GP + (f % spg) * FN
                    nc.tensor.matmul(
                        sub(f % spg), zeros_row[:], e_row[:, lo : lo + FN],
                        start=False, stop=False)
                # cross term
                if use_fp8:
                    for s in range(spg):
                        lo = g * GP + s * FN
                        nc.tensor.matmul(
                            sub(s), xT[:, :, m * P : (m + 1) * P],
                            yT[:, :, lo : lo + FN],
                            start=False, stop=True, perf_mode=DRMODE)
                else:
                    for c in range(kc):
                        for s in range(spg):
                            lo = g * GP + s * FN
                            nc.tensor.matmul(
                                sub(s), xT[:, c, m * P : (m + 1) * P],
                                yT[:, c, lo : lo + FN],
                                start=False, stop=(c == kc - 1))
                zch = zpool.tile([P, GP], f16)
                nc.scalar.activation(zch[:], ps[:], AF.Sqrt,
                                     bias=x2[:, m : m + 1])
                nc.vector.tensor_scalar(
                    zch[:], zch[:], -1.0, beta_b[:], ALU.mult, ALU.add)
                lo = g * GP
                nc.sync.dma_start(
                    z_d.ap()[m * P : (m + 1) * P, lo : lo + GP], zch[:])

    nc.compile()
    return nc


_CACHED = {}


def _get_nc():
    if "nc" not in _CACHED:
        _CACHED["nc"] = build_l2_kernel()
    return _CACHED["nc"]


def make_in_maps(X, Y, beta):
    in_maps = []
    for c in range(N_CORES):
        r, q = divmod(c, C_BLK)
        in_maps.append({
            "Xs": np.ascontiguousarray(
                X[r * ROWS_PER_CORE : (r + 1) * ROWS_PER_CORE]),
            "Y": np.ascontiguousarray(
                Y[:, q * COLS_PER_CORE : (q + 1) * COLS_PER_CORE]),
            "beta": beta,
        })
    return in_maps


def assemble(results):
    out = np.empty((N_ROW, N_COL), dtype=np.float32)
    for c in range(N_CORES):
        r, q = divmod(c, C_BLK)
        out[r * ROWS_PER_CORE : (r + 1) * ROWS_PER_CORE,
            q * COLS_PER_CORE : (q + 1) * COLS_PER_CORE] = (
            results[c]["Z"].astype(np.float32))
    return out


def kernel(X, Y, beta):
    X = np.ascontiguousarray(np.asarray(X, dtype=np.float32))
    Y = np.ascontiguousarray(np.asarray(Y, dtype=np.float32))
    beta = np.asarray(beta, dtype=np.float32).reshape(1, 1)
    assert X.shape == (N_ROW, RANK) and Y.shape == (RANK, N_COL)

    res = run_bass_kernel_spmd(_get_nc(), make_in_maps(X, Y, beta),
                               core_ids=list(range(N_CORES)))
    return assemble(res.results)


# revision 53
# speedup vs baseline: 29.1871x; 1.0163x over previous
"""Pairwise L2-distance kernel (retrieval_knn) for 8x Trainium2 NeuronCores.

Computes Z = beta - sqrt(max(||x||^2 + ||y||^2 - 2 X@Y, 0)) for
X:(8192,256) f32, Y:(256,8192) f32, beta:(1,) f32 -> Z:(8192,8192) f32.

Sharding: 2D grid, 4 row-blocks x 2 col-blocks. Core c handles
X rows [2048*(c//2), +2048) and Y cols [4096*(c%2), +4096), writing a
(2048, 4096) block of Z. Vs 1D row sharding this cuts per-core input
DMA from 9.4MB to 6.3MB and halves the Y-side prologue.

Per-core algorithm:
  - cross term via bf16 PE matmuls, K=256 as 2 chunks of 128, X scaled
    by -2 at convert time (exact power-of-two scale). bf16 over fp8
    DoubleRow is deliberate: every fp8-DR variant tried left the PE
    HAM clock gate throttled at 1.2 GHz for the whole main loop and
    measured 179-235us, while this bf16 structure holds 2.4 GHz and
    measures ~140us.
  - ||y||^2 injected into each PSUM accumulation group as one extra f16
    K=1 contraction row (ones (x) y2_f16), issued first (start=True).
  - main loop is g-outer/m-inner with 2048-wide groups; each LDWEIGHTS
    covers 4 back-to-back matmuls and 2 PSUM rounds are in flight —
    this exact structure measured HAM-warm; narrower variants throttle.
  - ~14 constant-dependent warm-up matmuls start the PE streaming at
    ~7us (before any DMA-gated work) so the clock gate is already warm
    when the real pipeline begins; only sustained (>3.4us) idleness
    re-throttles it.
  - Y-side streaming split into conv (ScalarE) / square (DVE) / column
    sum (PE) phases, hand-interleaved with the X-side transposes so no
    in-order engine queue head-of-line-blocks another engine's first
    need; the next column's strips are processed mid-column, borrowing
    PSUM rows of main-loop tiles (no extra PSUM banks).
  - ||x||^2 via Square+accum_out on the otherwise-idle ScalarE in the
    prologue, applied as the per-partition bias of the fused Sqrt
    activation reading 4 PSUM banks at once; fp16 output (tolerance is
    2e-2; fp16 adds ~3e-4).
  - z = beta - d in one VectorE tensor_scalar pass (fp16, 4x DVE mode).
  - output staged in per-group fp16 tiles, each stored with its own
    DMA; host converts to f32 and assembles the 2D blocks.
"""

from contextlib import ExitStack

import numpy as np

import concourse.bacc as bacc
import concourse.mybir as mybir
import concourse.tile as tile
from concourse.bass_utils import run_bass_kernel_spmd
from concourse.masks import make_identity

N_CORES = 8
N_ROW, RANK, N_COL = 8192, 256, 8192
R_BLK, C_BLK = 4, 2                    # core grid
ROWS_PER_CORE = N_ROW // R_BLK         # 2048
COLS_PER_CORE = N_COL // C_BLK         # 4096

P = 128      # partitions
FN = 512     # matmul free dim / PSUM bank (fp32)
CH = 1024    # Y-side strip width
GP = 2048    # main group width (4 PSUM banks; Act/DVE/store unit)

f32 = mybir.dt.float32
bf16 = mybir.dt.bfloat16
f16 = mybir.dt.float16
f8 = mybir.dt.float8e4

AF = mybir.ActivationFunctionType
ALU = mybir.AluOpType
DRMODE = mybir.MatmulPerfMode.DoubleRow


def build_l2_kernel(rows=ROWS_PER_CORE, rank=RANK, ncol=COLS_PER_CORE,
                    n_cores=N_CORES, use_fp8=False, fillers=0):
    """Build the per-core SPMD Bass program. Returns the compiled Bacc."""
    assert rows % P == 0 and rank % P == 0 and ncol % GP == 0
    mt = rows // P          # m-tiles (16)
    kc = rank // P          # k-chunks (2)
    ngp = ncol // GP        # groups per m row (2)
    spg = GP // FN          # 512-subtiles per group (4)

    nc = bacc.Bacc("TRN2", target_bir_lowering=False, debug=False,
                   num_devices=n_cores)

    xs_d = nc.dram_tensor("Xs", [rows, rank], f32, kind="ExternalInput")
    y_d = nc.dram_tensor("Y", [rank, ncol], f32, kind="ExternalInput")
    beta_d = nc.dram_tensor("beta", [1, 1], f32, kind="ExternalInput")
    # fp16 output block; host converts to f32.
    z_d = nc.dram_tensor("Z", [rows, ncol], f16, kind="ExternalOutput")

    with tile.TileContext(nc) as tc, ExitStack() as ctx:
        cpool = ctx.enter_context(tc.tile_pool(name="const", bufs=1))
        ypool = ctx.enter_context(tc.tile_pool(name="ybig", bufs=1))
        yf_pool = ctx.enter_context(tc.tile_pool(name="yf", bufs=8))
        ysq_pool = ctx.enter_context(tc.tile_pool(name="ysq", bufs=4))
        zpool = ctx.enter_context(tc.tile_pool(name="z", bufs=4))
        tp_stack = ExitStack()
        tp_psum = tp_stack.enter_context(
            tc.tile_pool(name="tpp", bufs=3, space="PSUM"))

        # ---- constants ----
        identity = cpool.tile([P, P], f32)
        make_identity(nc, identity[:])
        ones_row = cpool.tile([1, P], f16)       # lhsT of the y2-row matmul
        nc.gpsimd.memset(ones_row[:], 1.0)
        zeros_row = cpool.tile([1, P], f16)      # lhsT of filler matmuls
        nc.gpsimd.memset(zeros_row[:], 0.0)
        ones_col = cpool.tile([P, 1], bf16)      # lhsT of the y2 column-reduce
        nc.gpsimd.memset(ones_col[:], 1.0)
        beta_b = cpool.tile([P, 1], f32)
        b11 = cpool.tile([1, 1], f32)
        nc.sync.dma_start(b11[:], beta_d.ap()[:])
        nc.gpsimd.partition_broadcast(beta_b[:], b11[:])

        # ---- DMA order: Y strip 0 first (it gates the PE's first work),
        # then X, then the rest of Y. All loads upfront: the SP DMA queue
        # is in-order, so a load emitted after the main loop's stores
        # would not fire until every earlier store's data landed.
        yfs = {}

        def y_load(g):
            for c in range(kc):
                yf = yf_pool.tile([P, CH], f32, name="yf")
                nc.sync.dma_start(
                    yf[:], y_d.ap()[c * P : (c + 1) * P,
                                    g * CH : (g + 1) * CH])
                yfs[(g, c)] = yf

        y_load(0)
        # X in two halves so strip 1's load is not stuck behind the
        # whole 2MB slab: transposes(0,8) only need the first half
        xs_sb = cpool.tile([P, mt, rank], f32)
        nc.sync.dma_start(
            xs_sb[:, 0 : mt // 2, :],
            xs_d.ap()[0 : rows // 2].rearrange("(t p) k -> p t k", p=P))
        y_load(1)
        nc.sync.dma_start(
            xs_sb[:, mt // 2 :, :],
            xs_d.ap()[rows // 2 :].rearrange("(t p) k -> p t k", p=P))
        for g in range(2, ncol // CH):
            y_load(g)

        yT = ypool.tile([P, kc, ncol], f8 if use_fp8 else bf16,
                        name="yb", tag="yb")
        e_row = cpool.tile([1, ncol], f16)

        def y_conv(g):
            """bf16 conversion of strip g on ScalarE."""
            lo0 = g * CH
            for c in range(kc):
                nc.scalar.copy(yT[:, c, lo0 : lo0 + CH], yfs[(g, c)][:])

        def y_sq(g):
            """y^2 of strip g on the DVE (from the f32 staging tiles)."""
            sqs = []
            for c in range(kc):
                ysq = ysq_pool.tile([P, CH], bf16, name="ysq")
                nc.vector.tensor_tensor(
                    ysq[:], yfs[(g, c)][:], yfs[(g, c)][:], op=ALU.mult)
                sqs.append(ysq)
            return sqs

        def y_sum(g, sqs, y2ps):
            """Column-sum y^2 on the PE into the borrowed [1, CH] PSUM
            row (e_copy emitted separately)."""
            for c in range(kc):
                for s in range(CH // FN):
                    nc.tensor.matmul(
                        y2ps[0:1, s * FN : (s + 1) * FN], ones_col[:],
                        sqs[c][:, s * FN : (s + 1) * FN],
                        start=(c == 0), stop=(c == kc - 1))

        def e_copy(g, y2ps):
            nc.vector.tensor_copy(
                e_row[:, g * CH : (g + 1) * CH], y2ps[0:1, :])

        x2 = cpool.tile([P, mt], f32)
        xsq = cpool.tile([P, rank], f32)
        xT = cpool.tile([P, kc, rows], f8 if use_fp8 else bf16)

        # Emission order tuned so no engine's in-order queue blocks
        # another's first need:
        #  PE:  y2sum(s0) ~11us -> 32 transposes (DVE-paced) -> y2sum(s1)
        #       -> main matmuls, continuous => HAM warms once, stays warm
        #  Act: conv(s0) conv(s1) x2*16 -> sqrt(m0) ready ~13us
        #  DVE: ysq(s0) -> xT converts (paces transposes) -> ysq(s1) ->
        #       beta passes
        y2p_stack = ExitStack()
        y2_psum = y2p_stack.enter_context(
            tc.tile_pool(name="y2p", bufs=1, space="PSUM"))
        # [0:FN] warm-up scratch | [FN:FN+CH] strip-0 y2 | rest strip-1 y2
        # — disjoint regions so warm-up matmuls sprinkled between the
        # real prologue work never serialize against the y2 sums.
        y2t = y2_psum.tile([1, FN + 2 * CH], f32)

        # HAM warm-up: ~14 back-to-back matmuls that depend only on
        # constants, so the PE starts streaming at ~7us (right after
        # NEFF init) and the clock gate reaches 2.4 GHz before the real
        # work begins. Results land in a y2t slice that y_sum overwrites
        # (start=True) later. Only sustained (>3.4us) idleness
        # re-throttles, which the pipeline below avoids.
        warm_src = cpool.tile([P, FN], bf16)
        nc.vector.memset(warm_src[:], 0.0)

        def warm(n):
            for _ in range(n):
                nc.tensor.matmul(y2t[0:1, 0:FN], ones_col[:],
                                 warm_src[:], start=True, stop=True)

        warm(14)

        def transposes(m_lo, m_hi):
            for m in range(m_lo, m_hi):
                for c in range(kc):
                    pt = tp_psum.tile([P, P], f32, name="pt")
                    nc.tensor.transpose(
                        pt[:], xs_sb[:, m, c * P : (c + 1) * P],
                        identity[:])
                    nc.vector.tensor_scalar(
                        xT[:, c, m * P : (m + 1) * P], pt[:], -2.0, 0.0,
                        ALU.mult, ALU.add)

        # Emission order matched to DMA arrival times: strip 1's load
        # lands at ~16us (after the X slab), so its y^2 squares would
        # stall the DVE; the first 16 transposes' converts run during
        # that window instead, keeping every PE gap under the ~3.4us
        # re-throttle threshold.
        def x2_of(m):
            # x2 via Square+accum_out on ScalarE
            nc.scalar.activation(
                xsq[:], xs_sb[:, m, :], AF.Square,
                accum_out=x2[:, m : m + 1])

        y_conv(0)
        y_conv(1)
        s0 = y_sq(0)
        y_sum(0, s0, y2t[0:1, FN : FN + CH])
        warm(6)
        transposes(0, 8)
        warm(8)
        s1 = y_sq(1)
        e_copy(0, y2t[0:1, FN : FN + CH])
        y_sum(1, s1, y2t[0:1, FN + CH : FN + 2 * CH])
        warm(4)
        transposes(8, mt)
        e_copy(1, y2t[0:1, FN + CH : FN + 2 * CH])
        for m in range(mt):
            x2_of(m)
        y2p_stack.close()
        tp_stack.close()
        mm_psum = ctx.enter_context(
            tc.tile_pool(name="mmp", bufs=2, space="PSUM"))

        # ---- main loop: g-outer / m-inner. Column g's strips are ready
        # before it starts; the NEXT column's strips are processed early
        # (spread over m=2..6, borrowing PSUM rows at m=6) so the column
        # boundary has no Y-chain stall and the PE never idles.
        for g in range(ngp):
            for m in range(mt):
                ps = mm_psum.tile([P, GP], f32)
                if g + 1 < ngp:
                    if m == 2:
                        y_conv(2 * (g + 1))
                        nxt_s0 = y_sq(2 * (g + 1))
                    elif m == 4:
                        y_conv(2 * (g + 1) + 1)
                        nxt_s1 = y_sq(2 * (g + 1) + 1)
                    elif m == 6:
                        y_sum(2 * (g + 1), nxt_s0, ps[:, 0:CH])
                        e_copy(2 * (g + 1), ps[:, 0:CH])
                    elif m == 8:
                        y_sum(2 * (g + 1) + 1, nxt_s1, ps[:, 0:CH])
                        e_copy(2 * (g + 1) + 1, ps[:, 0:CH])

                def sub(s):
                    return ps[:, s * FN : (s + 1) * FN]

                # y2 row first (start=True); one LDWEIGHTS covers 4 MMs
                # (N=512 moving operands: the walrus lowering rejects
                # N=1024 with an s3d3_mm_num_elements ISA check)
                for s in range(spg):
                    lo = g * GP + s * FN
                    nc.tensor.matmul(
                        sub(s), ones_row[:], e_row[:, lo : lo + FN],
                        start=True, stop=False)
                # cross term, weight-batched per k-chunk
                if use_fp8:
                    for s in range(spg):
                        lo = g * GP + s * FN
                        nc.tensor.matmul(
                            sub(s), xT[:, :, m * P : (m + 1) * P],
                            yT[:, :, lo : lo + FN],
                            start=False, stop=True, perf_mode=DRMODE)
                else:
                    for c in range(kc):
                        for s in range(spg):
                            lo = g * GP + s * FN
                            nc.tensor.matmul(
                                sub(s), xT[:, c, m * P : (m + 1) * P],
                                yT[:, c, lo : lo + FN],
                                start=False, stop=(c == kc - 1))
                zch = zpool.tile([P, GP], f16)
                if g == ngp - 1 and m == mt - 1:
                    # last group: 512-wide pipelined post-processing so
                    # the kernel's exit chain (sqrt -> beta -> store)
                    # staggers instead of running serially at the end
                    for s in range(spg):
                        sl = slice(s * FN, (s + 1) * FN)
                        nc.scalar.activation(zch[:, sl], ps[:, sl],
                                             AF.Sqrt,
                                             bias=x2[:, m : m + 1])
                        nc.vector.tensor_scalar(
                            zch[:, sl], zch[:, sl], -1.0, beta_b[:],
                            ALU.mult, ALU.add)
                        lo = g * GP + s * FN
                        nc.sync.dma_start(
                            z_d.ap()[m * P : (m + 1) * P, lo : lo + FN],
                            zch[:, sl])
                else:
                    nc.scalar.activation(zch[:], ps[:], AF.Sqrt,
                                         bias=x2[:, m : m + 1])
                    nc.vector.tensor_scalar(
                        zch[:], zch[:], -1.0, beta_b[:],
                        ALU.mult, ALU.add)
                    lo = g * GP
                    nc.sync.dma_start(
                        z_d.ap()[m * P : (m + 1) * P, lo : lo + GP],
                        zch[:])

    nc.compile()
    return nc


_CACHED = {}


def _get_nc():
    if "nc" not in _CACHED:
        _CACHED["nc"] = build_l2_kernel()
    return _CACHED["nc"]


def make_in_maps(X, Y, beta):
    in_maps = []
    for c in range(N_CORES):
        r, q = divmod(c, C_BLK)
        in_maps.append({
            "Xs": np.ascontiguousarray(
                X[r * ROWS_PER_CORE : (r + 1) * ROWS_PER_CORE]),
            "Y": np.ascontiguousarray(
                Y[:, q * COLS_PER_CORE : (q + 1) * COLS_PER_CORE]),
            "beta": beta,
        })
    return in_maps


def assemble(results):
    out = np.empty((N_ROW, N_COL), dtype=np.float32)
    for c in range(N_CORES):
        r, q = divmod(c, C_BLK)
        out[r * ROWS_PER_CORE : (r + 1) * ROWS_PER_CORE,
            q * COLS_PER_CORE : (q + 1) * COLS_PER_CORE] = (
            results[c]["Z"].astype(np.float32))
    return out


def kernel(X, Y, beta):
    X = np.ascontiguousarray(np.asarray(X, dtype=np.float32))
    Y = np.ascontiguousarray(np.asarray(Y, dtype=np.float32))
    beta = np.asarray(beta, dtype=np.float32).reshape(1, 1)
    assert X.shape == (N_ROW, RANK) and Y.shape == (RANK, N_COL)

    res = run_bass_kernel_spmd(_get_nc(), make_in_maps(X, Y, beta),
                               core_ids=list(range(N_CORES)))
    return assemble(res.results)
